# revision 7
# baseline (speedup 1.0000x reference)
"""Trainium2 Bass kernel for nn_AttentionAggregator3d.

Math (per batch b):
    zmf = zm.reshape(CM, N)                     # N = D*W*H = 4096 tokens
    q = Wq @ zmf + bq ; k = Wk @ zmf + bk       # (16, N)
    v = Wv @ zmf + bv                           # (128, N)
    A = softmax_n(q^T k)                        # (N, N), softmax over keys n
    out = v @ A^T ; result = zc + gamma * out

Key transformations used by the kernel:
  * logits = zmf^T G zmf (+ col term) with G = Wq^T Wk precomputed on host,
    turning the K=16 contraction into a full K=128 PE contraction.
  * bq/bk only affect softmax through the per-key term r[n] = (Wk^T bq)·zm[:,n]
    (per-query terms cancel in softmax); handled as a per-partition exp bias.
  * Sharding: 8 cores = batch (2) x query-block (4, 1024 queries each). Each
    core sees its batch's zm rotated so its query block sits at columns 0:1024
    (softmax/PV sum over all keys, so key order is irrelevant).
  * Layout "B": exp'd scores E^T are kept (keys on partitions, queries free)
    so the PV matmul contracts over keys on the PE. Row sums for softmax are
    accumulated partly by PE ones-matmuls, partly by DVE adds (tunable split),
    then folded, inverted via exp(-ln s) (same ACT table set), broadcast with a
    K=1 matmul, and applied together with gamma / zc in a short DVE tail.
"""

import os
import sys
import types

import numpy as np

import concourse.bass as bass
import concourse.tile as tile
from concourse import bacc, mybir
from concourse.bass_utils import run_bass_kernel_spmd


def _ensure_ntff_hook() -> bool:
    """The grading image lacks antenv.axon_hooks; synthesize it from the
    boot module's ctypes NTFF driver so trace=True works under axon."""
    try:
        import antenv.axon_hooks  # noqa: F401

        return True
    except ImportError:
        pass
    try:
        import antenv
        from trn_agent_boot.trn_boot import _ntff_profile_via_ctypes

        hook = _ntff_profile_via_ctypes("/opt/axon/libaxon_pjrt.so")
        mod = types.ModuleType("antenv.axon_hooks")
        mod.get_axon_ntff_profile_hook = lambda: hook
        mod.set_axon_ntff_profile_hook = lambda h: None
        sys.modules["antenv.axon_hooks"] = mod
        antenv.axon_hooks = mod
        return hook is not None
    except Exception:
        return False

B, CC, CM, P = 2, 128, 128, 16
N = 16 * 16 * 16          # 4096 tokens
MBLK = N // 4             # 1024 queries per core
NCORES = 8
NCHUNK = N // 128         # 32 key chunks of 128

F32 = mybir.dt.float32
F32R = mybir.dt.float32r
AF = mybir.ActivationFunctionType
ALU = mybir.AluOpType

# chunks whose softmax-denominator contribution is summed on the PE
# (ones-matmul); the rest accumulate on the DVE. Tunable.
PE_SUM_PERIOD = int(os.environ.get("BASS_PE_SUM_PERIOD", "2"))

LAST_RESULTS = None  # BassKernelResults of the most recent run (for test.py)


def _r(ap):
    """float32r view of an fp32 AP: full-rate PE matmuls on TRN2."""
    return ap.bitcast(F32R)


def _build(use_qk_bias: bool):
    nc = bacc.Bacc(
        "TRN2",
        target_bir_lowering=False,
        debug=False,
        num_devices=NCORES,
    )

    zm_d = nc.dram_tensor("zm", (CM, N), F32R, kind="ExternalInput").ap()
    zc_d = nc.dram_tensor("zc", (CC, MBLK), F32, kind="ExternalInput").ap()
    gt_d = nc.dram_tensor("gt", (CM, CM), F32R, kind="ExternalInput").ap()
    wvt_d = nc.dram_tensor("wvt", (CM, CC), F32R, kind="ExternalInput").ap()
    gam_d = nc.dram_tensor("gam", (CC, 1), F32, kind="ExternalInput").ap()
    adv_d = nc.dram_tensor("adv", (CC, 1), F32, kind="ExternalInput").ap()
    onesc_d = nc.dram_tensor("onesc", (128, 1), F32R, kind="ExternalInput").ap()
    onesr_d = nc.dram_tensor("onesr", (1, 128), F32R, kind="ExternalInput").ap()
    if use_qk_bias:
        u_d = nc.dram_tensor("u", (CM, 1), F32R, kind="ExternalInput").ap()
    out_d = nc.dram_tensor("out", (CC, MBLK), F32, kind="ExternalOutput").ap()

    pe_sum = [j for j in range(NCHUNK) if j % PE_SUM_PERIOD == PE_SUM_PERIOD - 1]
    dve_sum = [j for j in range(NCHUNK) if j not in pe_sum]

    with tile.TileContext(nc) as tc:
        with (
            tc.tile_pool(name="consts", bufs=1) as consts,
            tc.tile_pool(name="epool", bufs=3) as epool,
            tc.tile_pool(name="lpool", bufs=2, space="PSUM") as lpool,
            tc.tile_pool(name="opool", bufs=1, space="PSUM") as opool,
            tc.tile_pool(name="spool", bufs=1, space="PSUM") as spool,
        ):
            zm_sb = consts.tile([CM, N], F32R, tag="zm")
            t_sb = consts.tile([CM, N], F32R, tag="t")
            vt_sb = consts.tile([128, N], F32R, tag="vt")  # chunk j at cols 128j (n_local, o)
            zc_sb = consts.tile([CC, MBLK], F32, tag="zc")
            gt_sb = consts.tile([CM, CM], F32R, tag="gt")
            wvt_sb = consts.tile([CM, CC], F32R, tag="wvt")
            gam_sb = consts.tile([CC, 1], F32, tag="gam")
            adv_sb = consts.tile([CC, 1], F32, tag="adv")
            ones_col = consts.tile([128, 1], F32R, tag="onesc")
            ones_row = consts.tile([1, 128], F32R, tag="onesr")
            acc = consts.tile([128, MBLK], F32R, tag="acc")
            lns = consts.tile([1, MBLK], F32, tag="lns")
            rvec = consts.tile([1, MBLK], F32R, tag="rvec")
            rb_sb = consts.tile([128, MBLK], F32, tag="rb")
            tmp_sb = consts.tile([CC, MBLK], F32, tag="tmp")
            out_sb = consts.tile([CC, MBLK], F32, tag="outsb")
            if use_qk_bias:
                u_sb = consts.tile([CM, 1], F32R, tag="u")
                rn_sb = consts.tile([128, NCHUNK], F32, tag="rn")

            # ---- input DMAs ----
            for i in range(4):
                nc.sync.dma_start(
                    zm_sb[:, i * MBLK : (i + 1) * MBLK],
                    zm_d[:, i * MBLK : (i + 1) * MBLK],
                )
            nc.sync.dma_start(gt_sb[:], gt_d)
            nc.sync.dma_start(wvt_sb[:], wvt_d)
            nc.sync.dma_start(zc_sb[:], zc_d)
            nc.sync.dma_start(gam_sb[:], gam_d)
            nc.sync.dma_start(adv_sb[:], adv_d)
            if use_qk_bias:
                nc.sync.dma_start(u_sb[:], u_d)
            nc.sync.dma_start(ones_col[:], onesc_d)
            nc.sync.dma_start(ones_row[:], onesr_d)

            out_ps = opool.tile([CC, MBLK], F32, tag="out")
            s_ps = spool.tile([1, MBLK], F32, tag="s")

            def emit_t_quarter(i):
                # t[:, 1024i:1024(i+1)] = G @ zm[:, ...]
                tps = lpool.tile([128, MBLK], F32, tag="L")
                for h in range(2):
                    c0 = i * MBLK + h * 512
                    nc.tensor.matmul(
                        tps[:, h * 512 : (h + 1) * 512],
                        gt_sb[:],
                        zm_sb[:, c0 : c0 + 512],
                        start=True,
                        stop=True,
                    )
                nc.vector.tensor_copy(
                    t_sb[:, i * MBLK : (i + 1) * MBLK], tps[:]
                )

            def emit_vt_quarter(i):
                # vt chunk j = (zm chunk j)^T @ Wv^T for j in 8i..8i+7
                vps = lpool.tile([128, MBLK], F32, tag="L")
                for k in range(8):
                    j = 8 * i + k
                    nc.tensor.matmul(
                        vps[:, 128 * k : 128 * (k + 1)],
                        zm_sb[:, 128 * j : 128 * (j + 1)],
                        wvt_sb[:],
                        start=True,
                        stop=True,
                    )
                nc.vector.tensor_copy(
                    vt_sb[:, i * MBLK : (i + 1) * MBLK], vps[:]
                )
                if use_qk_bias:
                    # r_n chunk j: (128,1) = (zm chunk j)^T @ u
                    rnps = lpool.tile([128, 8], F32, tag="L")
                    for k in range(8):
                        j = 8 * i + k
                        nc.tensor.matmul(
                            rnps[:, k : k + 1],
                            zm_sb[:, 128 * j : 128 * (j + 1)],
                            u_sb[:],
                            start=True,
                            stop=True,
                        )
                    nc.vector.tensor_copy(
                        rn_sb[:, 8 * i : 8 * (i + 1)], rnps[:]
                    )

            # t/vt for quarter 0 must precede the chunk loop; quarters 1-3
            # are emitted just-in-time from inside the loop so the PE never
            # idles waiting for phase boundaries.
            emit_t_quarter(0)
            emit_vt_quarter(0)

            e_tiles = {}
            first_pe = pe_sum[0] if pe_sum else None
            last_pe = pe_sum[-1] if pe_sum else None
            first_dve = dve_sum[0] if dve_sum else None

            for j in range(NCHUNK + 1):
                if j < NCHUNK:
                    if j % 8 == 1 and j // 8 + 1 <= 3:
                        emit_t_quarter(j // 8 + 1)
                    if j % 8 == 2 and j // 8 + 1 <= 3:
                        emit_vt_quarter(j // 8 + 1)
                    # logits^T chunk j: (keys 128, queries 1024)
                    lps = lpool.tile([128, MBLK], F32, tag="L")
                    for h in range(2):
                        nc.tensor.matmul(
                            lps[:, h * 512 : (h + 1) * 512],
                            t_sb[:, 128 * j : 128 * (j + 1)],
                            zm_sb[:, h * 512 : (h + 1) * 512],
                            start=True,
                            stop=True,
                        )
                    ej = epool.tile([128, MBLK], F32R, tag="E")
                    bias = rn_sb[:, j : j + 1] if use_qk_bias else 0.0
                    nc.scalar.activation(ej[:], lps[:], AF.Exp, bias=bias)
                    e_tiles[j] = ej
                if j >= 1:
                    jj = j - 1
                    ej = e_tiles.pop(jj)
                    for h in range(2):
                        nc.tensor.matmul(
                            out_ps[:, h * 512 : (h + 1) * 512],
                            vt_sb[:, 128 * jj : 128 * (jj + 1)],
                            ej[:, h * 512 : (h + 1) * 512],
                            start=(jj == 0),
                            stop=(jj == NCHUNK - 1),
                        )
                    if jj in pe_sum:
                        for h in range(2):
                            nc.tensor.matmul(
                                s_ps[:, h * 512 : (h + 1) * 512],
                                ones_col[:],
                                ej[:, h * 512 : (h + 1) * 512],
                                start=(jj == first_pe),
                                stop=False,
                            )
                    else:
                        if jj == first_dve:
                            nc.vector.tensor_copy(acc[:], ej[:])
                        else:
                            nc.vector.tensor_add(acc[:], acc[:], ej[:])

            # fold the DVE accumulator into s (cross-partition reduce on PE)
            for h in range(2):
                nc.tensor.matmul(
                    s_ps[:, h * 512 : (h + 1) * 512],
                    ones_col[:],
                    acc[:, h * 512 : (h + 1) * 512],
                    start=(first_pe is None),
                    stop=True,
                )

            # r = 1/s via exp(-ln s): stays in the natural_log_exp table set
            nc.scalar.activation(lns[:], s_ps[:], AF.Ln)
            nc.scalar.activation(rvec[:], lns[:], AF.Exp, scale=-1.0)

            # broadcast r across partitions with a K=1 matmul, fold gamma
            rb_ps = lpool.tile([128, MBLK], F32, tag="L")
            for h in range(2):
                nc.tensor.matmul(
                    rb_ps[:, h * 512 : (h + 1) * 512],
                    ones_row[:],
                    rvec[:, h * 512 : (h + 1) * 512],
                    start=True,
                    stop=True,
                )
            nc.vector.tensor_scalar(
                out=rb_sb[:],
                in0=rb_ps[:],
                scalar1=gam_sb[:, 0:1],
                scalar2=None,
                op0=ALU.mult,
            )
            # out = zc + (outPV * gamma/s + gamma*bv)
            nc.vector.tensor_tensor(tmp_sb[:], out_ps[:], rb_sb[:], op=ALU.mult)
            nc.vector.scalar_tensor_tensor(
                out_sb[:],
                tmp_sb[:],
                adv_sb[:, 0:1],
                zc_sb[:],
                op0=ALU.add,
                op1=ALU.add,
            )
            nc.sync.dma_start(out_d, out_sb[:])

    nc.compile()
    return nc


_CACHE = {}


def _get_program(use_qk_bias: bool):
    if use_qk_bias not in _CACHE:
        _CACHE[use_qk_bias] = _build(use_qk_bias)
    return _CACHE[use_qk_bias]


def kernel(zc, zm, Wq, bq, Wk, bk, Wv, bv, gamma):
    global LAST_RESULTS
    zc = np.ascontiguousarray(zc, dtype=np.float32)
    zm = np.ascontiguousarray(zm, dtype=np.float32)
    zmf = zm.reshape(B, CM, N)
    zcf = zc.reshape(B, CC, N)

    Wq = np.asarray(Wq, dtype=np.float32)
    Wk = np.asarray(Wk, dtype=np.float32)
    Wv = np.asarray(Wv, dtype=np.float32)
    gt = (Wk.astype(np.float64).T @ Wq.astype(np.float64)).astype(np.float32)
    wvt = np.ascontiguousarray(Wv.T, dtype=np.float32)
    gamma_v = np.float32(np.asarray(gamma).reshape(-1)[0])
    gam_arr = np.full((CC, 1), gamma_v, dtype=np.float32)
    adv_arr = (gamma_v * np.asarray(bv, dtype=np.float32)).reshape(CC, 1)
    adv_arr = np.ascontiguousarray(adv_arr)

    use_qk_bias = bool(np.any(bq)) or bool(np.any(bk))
    nc = _get_program(use_qk_bias)

    in_maps = []
    for c in range(NCORES):
        b, jblk = divmod(c, 4)
        m = {
            "zm": np.ascontiguousarray(np.roll(zmf[b], -MBLK * jblk, axis=1)),
            "zc": np.ascontiguousarray(zcf[b][:, MBLK * jblk : MBLK * (jblk + 1)]),
            "gt": gt,
            "wvt": wvt,
            "gam": gam_arr,
            "adv": adv_arr,
            "onesc": np.ones((128, 1), dtype=np.float32),
            "onesr": np.ones((1, 128), dtype=np.float32),
        }
        if use_qk_bias:
            m["u"] = np.ascontiguousarray(
                (Wk.T @ np.asarray(bq, dtype=np.float32)).reshape(CM, 1)
            )
        in_maps.append(m)

    trace = bool(int(os.environ.get("BASS_KERNEL_TRACE", "0")))
    if trace and not _ensure_ntff_hook():
        trace = False
    res = run_bass_kernel_spmd(
        nc,
        in_maps,
        core_ids=list(range(NCORES)),
        trace=trace,
    )
    LAST_RESULTS = res

    out = np.empty((B, CC, N), dtype=np.float32)
    for c in range(NCORES):
        b, jblk = divmod(c, 4)
        out[b][:, MBLK * jblk : MBLK * (jblk + 1)] = res.results[c]["out"]
    return out.reshape(zc.shape)
